# revision 10
# baseline (speedup 1.0000x reference)
"""BertSelfAttention (rotary, 16 heads, hd=64) on 8 trn2 cores.

Sharding: data-parallel over batch (4) x tensor-parallel over heads (2 groups
of 8). Core c handles batch c//2, head-group c%2. Each core computes its
heads' QKV projection, rotary, full attention, and writes ctx^T [512, 2048];
the host transposes/concatenates into the full [4, 2048, 1024] output.

v2 structure (vs v1):
 - Scores run as row-tiled 64x128 PE tiles: head A (d on partitions 0:64)
   and head B (64:128) stream concurrently, recovering the half-array loss
   of the hd=64 contraction. Four scores matmuls (2 j-chunks x 2 heads)
   fill a 4-bank PSUM tile.
 - One ACTIVATE exp covers all 4 banks [128, 2048] -> bf16 p tile, halving
   the per-call overhead of the scalar engine (the critical engine).
 - ctx keeps the v1 ones-augmented V (M=65): context and softmax
   denominator accumulate in the same stream. Col-tiled ctx was tried and
   abandoned: PSUM partitions 64-127 col tiles hit the quadrant-3 HW bug.
 - PE tile modes are phase-batched per j-pair (scores row-mode, then ctx
   full-mode) instead of alternating per matmul.
 - QKV+rotary chunks are interleaved as fillers two per attention unit.

Math notes (unchanged from v1):
 - scores_ref = (rot(q)/8 . rot(k))/8 = s_raw/64; 1/64 folded into the exp
   activation scale. |scores| <~ 0.8 so softmax needs no max-subtraction.
 - rotate_half runs on the PE as a signed permutation matmul (rsw).
 - Denominator via a ones-column appended to V (col 64 of each head block).
"""

import ml_dtypes
import numpy as np

import concourse.bass as bass
import concourse.tile as tile
from concourse import bacc, bass_utils, mybir

NPBF16 = ml_dtypes.bfloat16

B, S, H = 4, 2048, 1024
NH, HD = 16, 64
NCORES = 8
HPC = NH // 2            # heads per core = 8
NPAIR = HPC // 2         # head pairs per core = 4
DG = HPC * HD            # per-core head-dim group = 512
KC = H // 128            # contraction chunks = 8
IB = 512                 # attention i-block
NIB = S // IB            # 4
NJ = S // 128            # 16 j chunks

F32 = mybir.dt.float32
F32R = mybir.dt.float32r
BF16 = mybir.dt.bfloat16
EXP = mybir.ActivationFunctionType.Exp

_CACHE = {}


def _emit(nc, tc, ctx, ins, o_d):
    (xt_d, wq_d, wk_d, wv_d, bq_d, bk_d, bv_d, cos_d, sin_d, ones_d,
     rsw_d) = ins

    persist = ctx.enter_context(tc.tile_pool(name="persist", bufs=1))
    xt_sb = persist.tile([128, KC, S], BF16, tag="xt")
    wq_sb = persist.tile([128, KC, DG], BF16, tag="wq")
    wk_sb = persist.tile([128, KC, DG], BF16, tag="wk")
    wv_sb = persist.tile([128, KC, DG], BF16, tag="wv")
    cos_sb = persist.tile([128, S], BF16, tag="cos")
    sin_sb = persist.tile([128, S], F32R, tag="sin")
    qt = [persist.tile([128, S], BF16, tag=f"qt{p}", name=f"qt{p}")
          for p in range(NPAIR)]
    kt = [persist.tile([128, S], BF16, tag=f"kt{p}", name=f"kt{p}")
          for p in range(NPAIR)]
    vaug = [persist.tile([128, HPC * (HD + 1)], BF16, tag=f"va{j}",
                         name=f"va{j}")
            for j in range(NJ)]
    bq_sb = persist.tile([128, NPAIR], F32, tag="bq")
    bk_sb = persist.tile([128, NPAIR], F32, tag="bk")
    bv_sb = persist.tile([1, DG], BF16, tag="bv")
    ones_sb = persist.tile([128, IB], BF16, tag="ones")
    rsw_sb = persist.tile([128, 128], BF16, tag="rsw")

    # small/persistent inputs on the gpsimd (SWDGE) queue; big loads on sync
    nc.gpsimd.dma_start(bq_sb[:], bq_d)
    nc.gpsimd.dma_start(bk_sb[:], bk_d)
    nc.gpsimd.dma_start(bv_sb[:], bv_d)
    nc.gpsimd.dma_start(ones_sb[:], ones_d)
    nc.gpsimd.dma_start(rsw_sb[:], rsw_d)
    xt_r = xt_d.rearrange("(c p) i -> p c i", p=128)
    nc.sync.dma_start(wv_sb[:], wv_d.rearrange("(c p) d -> p c d", p=128))
    nc.sync.dma_start(xt_sb[:, :, 0:512], xt_r[:, :, 0:512])
    nc.sync.dma_start(wk_sb[:], wk_d.rearrange("(c p) d -> p c d", p=128))
    nc.sync.dma_start(wq_sb[:], wq_d.rearrange("(c p) d -> p c d", p=128))
    nc.sync.dma_start(cos_sb[:], cos_d)
    nc.sync.dma_start(sin_sb[:], sin_d.bitcast(F32R))
    for q in range(1, 4):
        qsl = slice(q * 512, (q + 1) * 512)
        nc.sync.dma_start(xt_sb[:, :, qsl], xt_r[:, :, qsl])
    ones128 = ones_sb[0:1, 0:128]

    tpool = ctx.enter_context(tc.tile_pool(name="tpool", bufs=4))
    q0pool = ctx.enter_context(tc.tile_pool(name="q0pool", bufs=3))
    qkps = ctx.enter_context(tc.tile_pool(name="qkps", bufs=2, space="PSUM"))
    scps = ctx.enter_context(tc.tile_pool(name="scps", bufs=2, space="PSUM"))
    cps = ctx.enter_context(tc.tile_pool(name="cps", bufs=2, space="PSUM"))
    ppool = ctx.enter_context(tc.tile_pool(name="ppool", bufs=6))
    capool = ctx.enter_context(tc.tile_pool(name="capool", bufs=2))
    rpool = ctx.enter_context(tc.tile_pool(name="rpool", bufs=2))
    bpool = ctx.enter_context(tc.tile_pool(name="bpool", bufs=2))
    npool = ctx.enter_context(tc.tile_pool(name="npool", bufs=2))

    def qk_phase1(which, pair, blk):
        """Q/K projection chunk -> biased q0 in SBUF (rotary in phase 2)."""
        w_sb, b_sb = (wq_sb, bq_sb) if which == "q" else (wk_sb, bk_sb)
        bsl = slice(blk * IB, (blk + 1) * IB)
        ps = qkps.tile([128, IB], F32, tag="qk", name="ps")
        for kc in range(KC):
            nc.tensor.matmul(
                ps[:], w_sb[:, kc, pair * 128:(pair + 1) * 128],
                xt_sb[:, kc, bsl], start=(kc == 0), stop=(kc == KC - 1))
        q0 = q0pool.tile([128, IB], BF16, tag="q0")
        nc.vector.tensor_scalar_add(q0[:], ps[:], b_sb[:, pair:pair + 1])
        return q0

    def qk_phase2(which, pair, blk, q0):
        """Rotary on a biased q0 chunk, into qt/kt."""
        out_t = qt[pair] if which == "q" else kt[pair]
        bsl = slice(blk * IB, (blk + 1) * IB)
        t2ps = qkps.tile([128, IB], F32, tag="qk", name="t2ps")
        nc.tensor.matmul(t2ps[:], rsw_sb[:], q0[:], start=True, stop=True)
        m1 = tpool.tile([128, IB], F32R, tag="m1")
        nc.vector.tensor_mul(m1[:], q0[:], cos_sb[:, bsl])
        t2s = tpool.tile([128, IB], F32R, tag="t2s")
        nc.vector.tensor_mul(t2s[:], t2ps[:].bitcast(F32R), sin_sb[:, bsl])
        nc.vector.tensor_add(out_t[:, bsl], m1[:], t2s[:])

    def qk_chunk(which, pair, blk):
        qk_phase2(which, pair, blk, qk_phase1(which, pair, blk))

    def v_chunk(jc):
        """V for all 8 heads at j-chunk jc, bias added, ones column."""
        vp = qkps.tile([128, DG], F32, tag="qk", name="vp")
        for kc in range(KC):
            nc.tensor.matmul(
                vp[:], xt_sb[:, kc, jc * 128:(jc + 1) * 128], wv_sb[:, kc, :],
                start=(kc == 0), stop=False)
        nc.tensor.matmul(vp[:], ones128, bv_sb[:], start=False, stop=True)
        vv = vaug[jc][:].rearrange("p (h c) -> p h c", h=HPC)
        nc.vector.tensor_copy(
            vv[:, :, HD:HD + 1],
            ones_sb[:, 0:HPC].rearrange("p (h one) -> p h one", one=1))
        nc.vector.tensor_copy(
            vv[:, :, 0:HD], vp[:].rearrange("p (h c) -> p h c", h=HPC))

    def unit(pair, ib, sched):
        """Attention for head pair `pair`, queries [ib*512, (ib+1)*512).

        One call per j-chunk: two row-tiled scores matmuls (heads A/B) into
        a 2-bank PSUM tile (pool bufs=2 double-buffers the ring), one
        ACTIVATE exp [128, 1024], then the lag-1 ctx matmuls for j-1.
        `sched` maps call index -> list of filler thunks (QKV production).
        """
        isl = slice(ib * IB, (ib + 1) * IB)
        ctx_ps = [cps.tile([HD + 1, IB], F32, tag="ctx", name=f"ctx{h}")
                  for h in range(2)]
        pend = []

        def emit_ctx():
            p, j = pend.pop(0)
            for h in range(2):
                hh = 2 * pair + h
                nc.tensor.matmul(
                    ctx_ps[h][:],
                    vaug[j][:, hh * (HD + 1):(hh + 1) * (HD + 1)],
                    p[:, h * IB:(h + 1) * IB],
                    start=(j == 0), stop=(j == NJ - 1))

        for j in range(NJ):
            sc = scps.tile([128, 2 * IB], F32, tag="sc", name="sc")
            for h in range(2):
                hp = slice(h * 64, (h + 1) * 64)
                nc.tensor.matmul(
                    sc[:, h * IB:(h + 1) * IB],
                    kt[pair][hp, j * 128:(j + 1) * 128], qt[pair][hp, isl],
                    start=True, stop=True)
            p = ppool.tile([128, 2 * IB], BF16, tag="p", name="p")
            nc.scalar.activation(p[:], sc[:], EXP, scale=1.0 / 64.0)
            pend.append((p, j))
            for th in sched.get(j, ()):
                th()
            # batch ctx four j-chunks at a time: fewer PE tile-mode switches
            if j >= 7 and j % 4 == 3:
                for _ in range(4):
                    emit_ctx()
        for th in sched.get(NJ, ()):
            th()
        for _ in range(4):
            emit_ctx()
        assert not pend
        for h in range(2):
            ca = capool.tile([HD + 1, IB], F32, tag="ca")
            nc.vector.tensor_copy(ca[:], ctx_ps[h][:])
            # custom DVE ops don't handle a shifted partition base: copy the
            # den row down to partition 0 before the reciprocal
            dn = rpool.tile([1, IB], F32, tag="dn")
            nc.vector.tensor_copy(dn[:], ca[HD:HD + 1, :])
            rec = rpool.tile([1, IB], F32, tag="rec")
            nc.vector.reciprocal_approx_fast(rec[:], dn[:])
            rbc = bpool.tile([HD, IB], F32, tag="rbc")
            nc.gpsimd.partition_broadcast(rbc[:], rec[:], channels=HD)
            ctxn = npool.tile([HD, IB], F32, tag="ctxn")
            nc.vector.tensor_mul(ctxn[:], ca[0:HD, :], rbc[:])
            hh = 2 * pair + h
            nc.sync.dma_start(o_d[hh * HD:(hh + 1) * HD, isl], ctxn[:])

    # minimal prefix: just enough for unit (pair0, ib0) to start streaming;
    # everything else is produced inside unit call slots
    v_chunk(0)
    v_chunk(1)
    qk_chunk("k", 0, 0)
    qk_chunk("q", 0, 0)

    def TH(*a):
        return lambda: qk_chunk(*a)

    def TV(jc):
        return lambda: v_chunk(jc)

    for pair in range(NPAIR):
        for ib in range(NIB):
            sched = {}
            if pair == 0 and ib == 0:
                sched = {0: [TV(2), TV(3)], 1: [TH("k", 0, 1), TV(4)],
                         2: [TV(5)], 3: [TV(6), TH("k", 0, 2)],
                         4: [TV(7)], 5: [TV(8)], 6: [TV(9)],
                         7: [TH("k", 0, 3), TV(10)], 8: [TV(11)],
                         9: [TV(12)], 10: [TV(13)], 11: [TV(14)],
                         12: [TV(15)], 13: [TH("q", 0, 1)]}
            elif ib == 0:
                sched = {1: [TH("k", pair, 1)], 3: [TH("k", pair, 2)],
                         7: [TH("k", pair, 3)], 10: [TH("q", pair, 1)]}
            elif ib == 1:
                sched = {5: [TH("q", pair, 2)]}
            elif ib == 2:
                sched = {5: [TH("q", pair, 3)]}
            elif ib == 3 and pair + 1 < NPAIR:
                sched = {4: [TH("k", pair + 1, 0)],
                         8: [TH("q", pair + 1, 0)]}
            unit(pair, ib, sched)

def _build():
    if "nc" in _CACHE:
        return _CACHE["nc"]
    nc = bacc.Bacc("TRN2", target_bir_lowering=False, debug=False,
                   num_devices=NCORES)
    names_shapes = [
        ("xt", [H, S], BF16), ("wq", [H, DG], BF16), ("wk", [H, DG], BF16),
        ("wv", [H, DG], BF16),
        ("bq", [128, NPAIR], F32), ("bk", [128, NPAIR], F32),
        ("bv", [1, DG], BF16),
        ("cos", [128, S], BF16), ("sin", [128, S], F32),
        ("ones", [128, IB], BF16), ("rsw", [128, 128], BF16),
    ]
    ins = [nc.dram_tensor(n, s, dt, kind="ExternalInput").ap()
           for n, s, dt in names_shapes]
    o_d = nc.dram_tensor("o", [DG, S], F32, kind="ExternalOutput").ap()
    from contextlib import ExitStack
    with tile.TileContext(nc) as tc:
        with ExitStack() as ctx:
            _emit(nc, tc, ctx, ins, o_d)
    nc.compile()
    _CACHE["nc"] = nc
    return nc


def _rotary_tables():
    inv_freq = (1.0 / (10000.0 ** (np.arange(0, HD, 2, dtype=np.float32)
                                   / np.float32(HD)))).astype(np.float32)
    t = np.arange(S, dtype=np.float32)
    freqs = np.outer(t, inv_freq).astype(np.float32)       # [S, 32]
    emb = np.concatenate([freqs, freqs], axis=-1)          # [S, 64]
    cos_t = np.cos(emb).T.astype(np.float32)               # [64, S]
    sin_t = np.sin(emb).T.astype(np.float32)               # unsigned
    cos2 = np.ascontiguousarray(np.concatenate([cos_t, cos_t], axis=0))
    sin2 = np.ascontiguousarray(np.concatenate([sin_t, sin_t], axis=0))
    # signed rotate-half permutation: t2_pre[d] = sign(d) * q[swap(d)],
    # sign = -1 on first half of each 64-block
    rsw = np.zeros((128, 128), dtype=np.float32)
    for d in range(128):
        blk, dd = d // 64, d % 64
        src = blk * 64 + (dd + 32) % 64
        rsw[src, d] = -1.0 if dd < 32 else 1.0
    return cos2, sin2, rsw


def _in_maps(hidden_states, Wq, bq, Wk, bk, Wv, bv):
    cos2, sin2, rsw = _rotary_tables()
    xts = [np.ascontiguousarray(hidden_states[b].T).astype(NPBF16)
           for b in range(B)]
    w_slices = {}
    for g in range(2):
        dsl = slice(g * DG, (g + 1) * DG)
        w_slices[g] = dict(
            wq=np.ascontiguousarray(Wq[:, dsl]).astype(NPBF16),
            wk=np.ascontiguousarray(Wk[:, dsl]).astype(NPBF16),
            wv=np.ascontiguousarray(Wv[:, dsl]).astype(NPBF16),
            bq=np.ascontiguousarray(bq[dsl].reshape(NPAIR, 128).T),
            bk=np.ascontiguousarray(bk[dsl].reshape(NPAIR, 128).T),
            bv=np.ascontiguousarray(bv[dsl].reshape(1, DG)).astype(NPBF16),
        )
    onesm = np.ones((128, IB), dtype=NPBF16)
    maps = []
    for c in range(NCORES):
        b, g = c // 2, c % 2
        m = {"xt": xts[b], "cos": cos2.astype(NPBF16), "sin": sin2,
             "ones": onesm, "rsw": rsw.astype(NPBF16)}
        m.update(w_slices[g])
        maps.append(m)
    return maps


def run(inputs, **kw):
    inputs = {k: np.asarray(v, dtype=np.float32) for k, v in inputs.items()}
    nc = _build()
    maps = _in_maps(**inputs)
    try:
        res = bass_utils.run_bass_kernel_spmd(
            nc, maps, core_ids=list(range(NCORES)), **kw)
    except Exception:
        # transient device errors (e.g. NRT_EXEC_UNIT_UNRECOVERABLE) clear on
        # retry
        res = bass_utils.run_bass_kernel_spmd(
            nc, maps, core_ids=list(range(NCORES)), **kw)
    out = np.empty((B, S, H), dtype=np.float32)
    for c in range(NCORES):
        b, g = c // 2, c % 2
        out[b, :, g * DG:(g + 1) * DG] = res.results[c]["o"].T
    return out, res


def kernel(**inputs):
    out, _ = run(inputs)
    return out


# revision 11
# speedup vs baseline: 1.0055x; 1.0055x over previous
"""BertSelfAttention (rotary, 16 heads, hd=64) on 8 trn2 cores.

Sharding: data-parallel over batch (4) x tensor-parallel over heads (2 groups
of 8). Core c handles batch c//2, head-group c%2. Each core computes its
heads' QKV projection, rotary, full attention, and writes ctx^T [512, 2048];
the host transposes/concatenates into the full [4, 2048, 1024] output.

v2 structure (vs v1):
 - Scores run as row-tiled 64x128 PE tiles: head A (d on partitions 0:64)
   and head B (64:128) stream concurrently, recovering the half-array loss
   of the hd=64 contraction. Four scores matmuls (2 j-chunks x 2 heads)
   fill a 4-bank PSUM tile.
 - One ACTIVATE exp covers all 4 banks [128, 2048] -> bf16 p tile, halving
   the per-call overhead of the scalar engine (the critical engine).
 - ctx keeps the v1 ones-augmented V (M=65): context and softmax
   denominator accumulate in the same stream. Col-tiled ctx was tried and
   abandoned: PSUM partitions 64-127 col tiles hit the quadrant-3 HW bug.
 - PE tile modes are phase-batched per j-pair (scores row-mode, then ctx
   full-mode) instead of alternating per matmul.
 - QKV+rotary chunks are interleaved as fillers two per attention unit.

Math notes (unchanged from v1):
 - scores_ref = (rot(q)/8 . rot(k))/8 = s_raw/64; 1/64 folded into the exp
   activation scale. |scores| <~ 0.8 so softmax needs no max-subtraction.
 - rotate_half runs on the PE as a signed permutation matmul (rsw).
 - Denominator via a ones-column appended to V (col 64 of each head block).
"""

import ml_dtypes
import numpy as np

import concourse.bass as bass
import concourse.tile as tile
from concourse import bacc, bass_utils, mybir

NPBF16 = ml_dtypes.bfloat16

B, S, H = 4, 2048, 1024
NH, HD = 16, 64
NCORES = 8
HPC = NH // 2            # heads per core = 8
NPAIR = HPC // 2         # head pairs per core = 4
DG = HPC * HD            # per-core head-dim group = 512
KC = H // 128            # contraction chunks = 8
IB = 512                 # attention i-block
NIB = S // IB            # 4
NJ = S // 128            # 16 j chunks

F32 = mybir.dt.float32
F32R = mybir.dt.float32r
BF16 = mybir.dt.bfloat16
EXP = mybir.ActivationFunctionType.Exp

_CACHE = {}


def _emit(nc, tc, ctx, ins, o_d):
    (xt_d, wq_d, wk_d, wv_d, bq_d, bk_d, bv_d, cos_d, sin_d, ones_d,
     rsw_d) = ins

    persist = ctx.enter_context(tc.tile_pool(name="persist", bufs=1))
    xt_sb = persist.tile([128, KC, S], BF16, tag="xt")
    wq_sb = persist.tile([128, KC, DG], BF16, tag="wq")
    wk_sb = persist.tile([128, KC, DG], BF16, tag="wk")
    wv_sb = persist.tile([128, KC, DG], BF16, tag="wv")
    cos_sb = persist.tile([128, S], BF16, tag="cos")
    sin_sb = persist.tile([128, S], F32R, tag="sin")
    qt = [persist.tile([128, S], BF16, tag=f"qt{p}", name=f"qt{p}")
          for p in range(NPAIR)]
    kt = [persist.tile([128, S], BF16, tag=f"kt{p}", name=f"kt{p}")
          for p in range(NPAIR)]
    vaug = [persist.tile([128, HPC * (HD + 1)], BF16, tag=f"va{j}",
                         name=f"va{j}")
            for j in range(NJ)]
    bq_sb = persist.tile([128, NPAIR], F32, tag="bq")
    bk_sb = persist.tile([128, NPAIR], F32, tag="bk")
    bv_sb = persist.tile([1, DG], BF16, tag="bv")
    ones_sb = persist.tile([128, IB], BF16, tag="ones")
    rsw_sb = persist.tile([128, 128], BF16, tag="rsw")

    # small/persistent inputs on the gpsimd (SWDGE) queue; big loads on sync
    nc.gpsimd.dma_start(bq_sb[:], bq_d)
    nc.gpsimd.dma_start(bk_sb[:], bk_d)
    nc.gpsimd.dma_start(bv_sb[:], bv_d)
    nc.gpsimd.dma_start(ones_sb[:], ones_d)
    nc.gpsimd.dma_start(rsw_sb[:], rsw_d)
    xt_r = xt_d.rearrange("(c p) i -> p c i", p=128)
    nc.sync.dma_start(wv_sb[:], wv_d.rearrange("(c p) d -> p c d", p=128))
    nc.sync.dma_start(xt_sb[:, :, 0:512], xt_r[:, :, 0:512])
    nc.sync.dma_start(wk_sb[:], wk_d.rearrange("(c p) d -> p c d", p=128))
    nc.sync.dma_start(wq_sb[:], wq_d.rearrange("(c p) d -> p c d", p=128))
    nc.sync.dma_start(cos_sb[:], cos_d)
    nc.sync.dma_start(sin_sb[:], sin_d.bitcast(F32R))
    for q in range(1, 4):
        qsl = slice(q * 512, (q + 1) * 512)
        nc.sync.dma_start(xt_sb[:, :, qsl], xt_r[:, :, qsl])
    ones128 = ones_sb[0:1, 0:128]

    tpool = ctx.enter_context(tc.tile_pool(name="tpool", bufs=4))
    q0pool = ctx.enter_context(tc.tile_pool(name="q0pool", bufs=3))
    qkps = ctx.enter_context(tc.tile_pool(name="qkps", bufs=2, space="PSUM"))
    scps = ctx.enter_context(tc.tile_pool(name="scps", bufs=2, space="PSUM"))
    cps = ctx.enter_context(tc.tile_pool(name="cps", bufs=2, space="PSUM"))
    ppool = ctx.enter_context(tc.tile_pool(name="ppool", bufs=6))
    capool = ctx.enter_context(tc.tile_pool(name="capool", bufs=2))
    rpool = ctx.enter_context(tc.tile_pool(name="rpool", bufs=2))
    bpool = ctx.enter_context(tc.tile_pool(name="bpool", bufs=2))
    npool = ctx.enter_context(tc.tile_pool(name="npool", bufs=2))

    def qk_phase1(which, pair, blk):
        """Q/K projection chunk -> biased q0 in SBUF (rotary in phase 2)."""
        w_sb, b_sb = (wq_sb, bq_sb) if which == "q" else (wk_sb, bk_sb)
        bsl = slice(blk * IB, (blk + 1) * IB)
        ps = qkps.tile([128, IB], F32, tag="qk", name="ps")
        for kc in range(KC):
            nc.tensor.matmul(
                ps[:], w_sb[:, kc, pair * 128:(pair + 1) * 128],
                xt_sb[:, kc, bsl], start=(kc == 0), stop=(kc == KC - 1))
        q0 = q0pool.tile([128, IB], BF16, tag="q0")
        nc.vector.tensor_scalar_add(q0[:], ps[:], b_sb[:, pair:pair + 1])
        return q0

    def qk_phase2(which, pair, blk, q0):
        """Rotary on a biased q0 chunk, into qt/kt."""
        out_t = qt[pair] if which == "q" else kt[pair]
        bsl = slice(blk * IB, (blk + 1) * IB)
        t2ps = qkps.tile([128, IB], F32, tag="qk", name="t2ps")
        nc.tensor.matmul(t2ps[:], rsw_sb[:], q0[:], start=True, stop=True)
        m1 = tpool.tile([128, IB], F32R, tag="m1")
        nc.vector.tensor_mul(m1[:], q0[:], cos_sb[:, bsl])
        t2s = tpool.tile([128, IB], F32R, tag="t2s")
        nc.vector.tensor_mul(t2s[:], t2ps[:].bitcast(F32R), sin_sb[:, bsl])
        nc.vector.tensor_add(out_t[:, bsl], m1[:], t2s[:])

    def qk_chunk(which, pair, blk):
        qk_phase2(which, pair, blk, qk_phase1(which, pair, blk))

    def v_chunk(jc):
        """V for all 8 heads at j-chunk jc, bias added, ones column."""
        vp = qkps.tile([128, DG], F32, tag="qk", name="vp")
        for kc in range(KC):
            nc.tensor.matmul(
                vp[:], xt_sb[:, kc, jc * 128:(jc + 1) * 128], wv_sb[:, kc, :],
                start=(kc == 0), stop=False)
        nc.tensor.matmul(vp[:], ones128, bv_sb[:], start=False, stop=True)
        vv = vaug[jc][:].rearrange("p (h c) -> p h c", h=HPC)
        nc.vector.tensor_copy(
            vv[:, :, HD:HD + 1],
            ones_sb[:, 0:HPC].rearrange("p (h one) -> p h one", one=1))
        nc.vector.tensor_copy(
            vv[:, :, 0:HD], vp[:].rearrange("p (h c) -> p h c", h=HPC))

    def unit(pair, ib, sched):
        """Attention for head pair `pair`, queries [ib*512, (ib+1)*512).

        One call per j-chunk: two row-tiled scores matmuls (heads A/B) into
        a 2-bank PSUM tile (pool bufs=2 double-buffers the ring), one
        ACTIVATE exp [128, 1024], then the lag-1 ctx matmuls for j-1.
        `sched` maps call index -> list of filler thunks (QKV production).
        """
        isl = slice(ib * IB, (ib + 1) * IB)
        ctx_ps = [cps.tile([HD + 1, IB], F32, tag="ctx", name=f"ctx{h}")
                  for h in range(2)]
        pend = []

        def emit_ctx():
            p, j = pend.pop(0)
            for h in range(2):
                hh = 2 * pair + h
                nc.tensor.matmul(
                    ctx_ps[h][:],
                    vaug[j][:, hh * (HD + 1):(hh + 1) * (HD + 1)],
                    p[:, h * IB:(h + 1) * IB],
                    start=(j == 0), stop=(j == NJ - 1))

        for j in range(NJ):
            sc = scps.tile([128, 2 * IB], F32, tag="sc", name="sc")
            for h in range(2):
                hp = slice(h * 64, (h + 1) * 64)
                nc.tensor.matmul(
                    sc[:, h * IB:(h + 1) * IB],
                    kt[pair][hp, j * 128:(j + 1) * 128], qt[pair][hp, isl],
                    start=True, stop=True)
            p = ppool.tile([128, 2 * IB], BF16, tag="p", name="p")
            nc.scalar.activation(p[:], sc[:], EXP, scale=1.0 / 64.0)
            pend.append((p, j))
            for th in sched.get(j, ()):
                th()
            # batch ctx two j-chunks at a time: fewer PE tile-mode switches
            if j >= 3 and j % 2 == 1:
                emit_ctx()
                emit_ctx()
        for th in sched.get(NJ, ()):
            th()
        emit_ctx()
        emit_ctx()
        assert not pend
        for h in range(2):
            ca = capool.tile([HD + 1, IB], F32, tag="ca")
            nc.vector.tensor_copy(ca[:], ctx_ps[h][:])
            # custom DVE ops don't handle a shifted partition base: copy the
            # den row down to partition 0 before the reciprocal
            dn = rpool.tile([1, IB], F32, tag="dn")
            nc.vector.tensor_copy(dn[:], ca[HD:HD + 1, :])
            rec = rpool.tile([1, IB], F32, tag="rec")
            nc.vector.reciprocal_approx_fast(rec[:], dn[:])
            rbc = bpool.tile([HD, IB], F32, tag="rbc")
            nc.gpsimd.partition_broadcast(rbc[:], rec[:], channels=HD)
            ctxn = npool.tile([HD, IB], F32, tag="ctxn")
            nc.vector.tensor_mul(ctxn[:], ca[0:HD, :], rbc[:])
            hh = 2 * pair + h
            nc.sync.dma_start(o_d[hh * HD:(hh + 1) * HD, isl], ctxn[:])

    # minimal prefix: just enough for unit (pair0, ib0) to start streaming;
    # everything else is produced inside unit call slots
    v_chunk(0)
    v_chunk(1)
    qk_chunk("k", 0, 0)
    qk_chunk("q", 0, 0)

    def TH(*a):
        return lambda: qk_chunk(*a)

    def TV(jc):
        return lambda: v_chunk(jc)

    for pair in range(NPAIR):
        for ib in range(NIB):
            sched = {}
            if pair == 0 and ib == 0:
                sched = {0: [TV(2), TV(3)], 1: [TH("k", 0, 1), TV(4)],
                         2: [TV(5)], 3: [TV(6), TH("k", 0, 2)],
                         4: [TV(7)], 5: [TV(8)], 6: [TV(9)],
                         7: [TH("k", 0, 3), TV(10)], 8: [TV(11)],
                         9: [TV(12)], 10: [TV(13)], 11: [TV(14)],
                         12: [TV(15)], 13: [TH("q", 0, 1)]}
            elif ib == 0:
                sched = {1: [TH("k", pair, 1)], 3: [TH("k", pair, 2)],
                         7: [TH("k", pair, 3)], 10: [TH("q", pair, 1)]}
            elif ib == 1:
                sched = {5: [TH("q", pair, 2)]}
            elif ib == 2:
                sched = {5: [TH("q", pair, 3)]}
            elif ib == 3 and pair + 1 < NPAIR:
                sched = {4: [TH("k", pair + 1, 0)],
                         8: [TH("q", pair + 1, 0)]}
            unit(pair, ib, sched)

def _build():
    if "nc" in _CACHE:
        return _CACHE["nc"]
    nc = bacc.Bacc("TRN2", target_bir_lowering=False, debug=False,
                   num_devices=NCORES)
    names_shapes = [
        ("xt", [H, S], BF16), ("wq", [H, DG], BF16), ("wk", [H, DG], BF16),
        ("wv", [H, DG], BF16),
        ("bq", [128, NPAIR], F32), ("bk", [128, NPAIR], F32),
        ("bv", [1, DG], BF16),
        ("cos", [128, S], BF16), ("sin", [128, S], F32),
        ("ones", [128, IB], BF16), ("rsw", [128, 128], BF16),
    ]
    ins = [nc.dram_tensor(n, s, dt, kind="ExternalInput").ap()
           for n, s, dt in names_shapes]
    o_d = nc.dram_tensor("o", [DG, S], F32, kind="ExternalOutput").ap()
    from contextlib import ExitStack
    with tile.TileContext(nc) as tc:
        with ExitStack() as ctx:
            _emit(nc, tc, ctx, ins, o_d)
    nc.compile()
    _CACHE["nc"] = nc
    return nc


def _rotary_tables():
    inv_freq = (1.0 / (10000.0 ** (np.arange(0, HD, 2, dtype=np.float32)
                                   / np.float32(HD)))).astype(np.float32)
    t = np.arange(S, dtype=np.float32)
    freqs = np.outer(t, inv_freq).astype(np.float32)       # [S, 32]
    emb = np.concatenate([freqs, freqs], axis=-1)          # [S, 64]
    cos_t = np.cos(emb).T.astype(np.float32)               # [64, S]
    sin_t = np.sin(emb).T.astype(np.float32)               # unsigned
    cos2 = np.ascontiguousarray(np.concatenate([cos_t, cos_t], axis=0))
    sin2 = np.ascontiguousarray(np.concatenate([sin_t, sin_t], axis=0))
    # signed rotate-half permutation: t2_pre[d] = sign(d) * q[swap(d)],
    # sign = -1 on first half of each 64-block
    rsw = np.zeros((128, 128), dtype=np.float32)
    for d in range(128):
        blk, dd = d // 64, d % 64
        src = blk * 64 + (dd + 32) % 64
        rsw[src, d] = -1.0 if dd < 32 else 1.0
    return cos2, sin2, rsw


def _in_maps(hidden_states, Wq, bq, Wk, bk, Wv, bv):
    cos2, sin2, rsw = _rotary_tables()
    xts = [np.ascontiguousarray(hidden_states[b].T).astype(NPBF16)
           for b in range(B)]
    w_slices = {}
    for g in range(2):
        dsl = slice(g * DG, (g + 1) * DG)
        w_slices[g] = dict(
            wq=np.ascontiguousarray(Wq[:, dsl]).astype(NPBF16),
            wk=np.ascontiguousarray(Wk[:, dsl]).astype(NPBF16),
            wv=np.ascontiguousarray(Wv[:, dsl]).astype(NPBF16),
            bq=np.ascontiguousarray(bq[dsl].reshape(NPAIR, 128).T),
            bk=np.ascontiguousarray(bk[dsl].reshape(NPAIR, 128).T),
            bv=np.ascontiguousarray(bv[dsl].reshape(1, DG)).astype(NPBF16),
        )
    onesm = np.ones((128, IB), dtype=NPBF16)
    maps = []
    for c in range(NCORES):
        b, g = c // 2, c % 2
        m = {"xt": xts[b], "cos": cos2.astype(NPBF16), "sin": sin2,
             "ones": onesm, "rsw": rsw.astype(NPBF16)}
        m.update(w_slices[g])
        maps.append(m)
    return maps


def run(inputs, **kw):
    inputs = {k: np.asarray(v, dtype=np.float32) for k, v in inputs.items()}
    nc = _build()
    maps = _in_maps(**inputs)
    try:
        res = bass_utils.run_bass_kernel_spmd(
            nc, maps, core_ids=list(range(NCORES)), **kw)
    except Exception:
        # transient device errors (e.g. NRT_EXEC_UNIT_UNRECOVERABLE) clear on
        # retry
        res = bass_utils.run_bass_kernel_spmd(
            nc, maps, core_ids=list(range(NCORES)), **kw)
    out = np.empty((B, S, H), dtype=np.float32)
    for c in range(NCORES):
        b, g = c // 2, c % 2
        out[b, :, g * DG:(g + 1) * DG] = res.results[c]["o"].T
    return out, res


def kernel(**inputs):
    out, _ = run(inputs)
    return out


# revision 12
# speedup vs baseline: 1.0371x; 1.0315x over previous
"""BertSelfAttention (rotary, 16 heads, hd=64) on 8 trn2 cores.

Sharding: data-parallel over batch (4) x tensor-parallel over heads (2 groups
of 8). Core c handles batch c//2, head-group c%2. Each core computes its
heads' QKV projection, rotary, full attention, and writes ctx^T [512, 2048];
the host transposes/concatenates into the full [4, 2048, 1024] output.

v2 structure (vs v1):
 - Scores run as row-tiled 64x128 PE tiles: head A (d on partitions 0:64)
   and head B (64:128) stream concurrently, recovering the half-array loss
   of the hd=64 contraction. Four scores matmuls (2 j-chunks x 2 heads)
   fill a 4-bank PSUM tile.
 - One ACTIVATE exp covers all 4 banks [128, 2048] -> bf16 p tile, halving
   the per-call overhead of the scalar engine (the critical engine).
 - ctx keeps the v1 ones-augmented V (M=65): context and softmax
   denominator accumulate in the same stream. Col-tiled ctx was tried and
   abandoned: PSUM partitions 64-127 col tiles hit the quadrant-3 HW bug.
 - PE tile modes are phase-batched per j-pair (scores row-mode, then ctx
   full-mode) instead of alternating per matmul.
 - QKV+rotary chunks are interleaved as fillers two per attention unit.

Math notes (unchanged from v1):
 - scores_ref = (rot(q)/8 . rot(k))/8 = s_raw/64; 1/64 folded into the exp
   activation scale. |scores| <~ 0.8 so softmax needs no max-subtraction.
 - rotate_half runs on the PE as a signed permutation matmul (rsw).
 - Denominator via a ones-column appended to V (col 64 of each head block).
"""

import ml_dtypes
import numpy as np

import concourse.bass as bass
import concourse.tile as tile
from concourse import bacc, bass_utils, mybir

NPBF16 = ml_dtypes.bfloat16
NPF8 = ml_dtypes.float8_e4m3

B, S, H = 4, 2048, 1024
NH, HD = 16, 64
NCORES = 8
HPC = NH // 2            # heads per core = 8
NPAIR = HPC // 2         # head pairs per core = 4
DG = HPC * HD            # per-core head-dim group = 512
KC = H // 128            # contraction chunks = 8
IB = 512                 # attention i-block
NIB = S // IB            # 4
NJ = S // 128            # 16 j chunks

F32 = mybir.dt.float32
F8 = mybir.dt.float8e4
F32R = mybir.dt.float32r
BF16 = mybir.dt.bfloat16
EXP = mybir.ActivationFunctionType.Exp

_CACHE = {}


def _emit(nc, tc, ctx, ins, o_d):
    (xt_d, xt8_d, wq8_d, wk8_d, wv_d, bq_d, bk_d, bv_d, cos_d, sin_d,
     ones_d, rsw_d) = ins

    persist = ctx.enter_context(tc.tile_pool(name="persist", bufs=1))
    xt_sb = persist.tile([128, KC, S], BF16, tag="xt")
    xt8_sb = persist.tile([128, KC, S], F8, tag="xt8")
    wq_sb = persist.tile([128, KC, DG], F8, tag="wq")
    wk_sb = persist.tile([128, KC, DG], F8, tag="wk")
    wv_sb = persist.tile([128, KC, DG], BF16, tag="wv")
    cos_sb = persist.tile([128, S], BF16, tag="cos")
    sin_sb = persist.tile([128, S], F32R, tag="sin")
    qt = [persist.tile([128, S], BF16, tag=f"qt{p}", name=f"qt{p}")
          for p in range(NPAIR)]
    kt = [persist.tile([128, S], BF16, tag=f"kt{p}", name=f"kt{p}")
          for p in range(NPAIR)]
    vaug = [persist.tile([128, HPC * (HD + 1)], BF16, tag=f"va{j}",
                         name=f"va{j}")
            for j in range(NJ)]
    bq_sb = persist.tile([128, NPAIR], F32, tag="bq")
    bk_sb = persist.tile([128, NPAIR], F32, tag="bk")
    bv_sb = persist.tile([1, DG], BF16, tag="bv")
    ones_sb = persist.tile([128, IB], BF16, tag="ones")
    rsw_sb = persist.tile([128, 128], BF16, tag="rsw")

    # small/persistent inputs on the gpsimd (SWDGE) queue; big loads on sync
    nc.gpsimd.dma_start(bq_sb[:], bq_d)
    nc.gpsimd.dma_start(bk_sb[:], bk_d)
    nc.gpsimd.dma_start(bv_sb[:], bv_d)
    nc.gpsimd.dma_start(ones_sb[:], ones_d)
    nc.gpsimd.dma_start(rsw_sb[:], rsw_d)
    xt_r = xt_d.rearrange("(c p) i -> p c i", p=128)
    xt8_r = xt8_d.rearrange("(c p) i -> p c i", p=128)
    nc.sync.dma_start(wv_sb[:], wv_d.rearrange("(c p) d -> p c d", p=128))
    nc.sync.dma_start(xt_sb[:, :, 0:512], xt_r[:, :, 0:512])
    nc.sync.dma_start(wk_sb[:], wk8_d.rearrange("(c p) d -> p c d", p=128))
    nc.sync.dma_start(xt8_sb[:, :, 0:512], xt8_r[:, :, 0:512])
    nc.sync.dma_start(wq_sb[:], wq8_d.rearrange("(c p) d -> p c d", p=128))
    nc.sync.dma_start(cos_sb[:], cos_d)
    nc.sync.dma_start(sin_sb[:], sin_d.bitcast(F32R))
    for q in range(1, 4):
        qsl = slice(q * 512, (q + 1) * 512)
        nc.sync.dma_start(xt_sb[:, :, qsl], xt_r[:, :, qsl])
        nc.sync.dma_start(xt8_sb[:, :, qsl], xt8_r[:, :, qsl])
    ones128 = ones_sb[0:1, 0:128]

    tpool = ctx.enter_context(tc.tile_pool(name="tpool", bufs=4))
    q0pool = ctx.enter_context(tc.tile_pool(name="q0pool", bufs=3))
    qkps = ctx.enter_context(tc.tile_pool(name="qkps", bufs=2, space="PSUM"))
    scps = ctx.enter_context(tc.tile_pool(name="scps", bufs=2, space="PSUM"))
    cps = ctx.enter_context(tc.tile_pool(name="cps", bufs=2, space="PSUM"))
    ppool = ctx.enter_context(tc.tile_pool(name="ppool", bufs=6))
    capool = ctx.enter_context(tc.tile_pool(name="capool", bufs=2))
    rpool = ctx.enter_context(tc.tile_pool(name="rpool", bufs=2))
    bpool = ctx.enter_context(tc.tile_pool(name="bpool", bufs=2))
    npool = ctx.enter_context(tc.tile_pool(name="npool", bufs=2))

    def qk_phase1(which, pair, blk):
        """Q/K projection chunk -> biased q0 in SBUF (rotary in phase 2)."""
        w_sb, b_sb = (wq_sb, bq_sb) if which == "q" else (wk_sb, bk_sb)
        bsl = slice(blk * IB, (blk + 1) * IB)
        ps = qkps.tile([128, IB], F32, tag="qk", name="ps")
        # fp8 DoubleRow: contraction pairs (2c,2c+1) pack 2 rows/cell; W and
        # x are pre-scaled by 32 on the host (fp8 dynamic range), compensated
        # in the exp scale (1/64 -> 1/65536)
        for c in range(KC // 2):
            nc.tensor.matmul(
                ps[:], w_sb[:, 2 * c:2 * c + 2, pair * 128:(pair + 1) * 128],
                xt8_sb[:, 2 * c:2 * c + 2, bsl],
                perf_mode=mybir.MatmulPerfMode.DoubleRow,
                start=(c == 0), stop=(c == KC // 2 - 1))
        q0 = q0pool.tile([128, IB], BF16, tag="q0")
        nc.vector.tensor_scalar_add(q0[:], ps[:], b_sb[:, pair:pair + 1])
        return q0

    def qk_phase2(which, pair, blk, q0):
        """Rotary on a biased q0 chunk, into qt/kt."""
        out_t = qt[pair] if which == "q" else kt[pair]
        bsl = slice(blk * IB, (blk + 1) * IB)
        t2ps = qkps.tile([128, IB], F32, tag="qk", name="t2ps")
        nc.tensor.matmul(t2ps[:], rsw_sb[:], q0[:], start=True, stop=True)
        m1 = tpool.tile([128, IB], F32R, tag="m1")
        nc.vector.tensor_mul(m1[:], q0[:], cos_sb[:, bsl])
        t2s = tpool.tile([128, IB], F32R, tag="t2s")
        nc.vector.tensor_mul(t2s[:], t2ps[:].bitcast(F32R), sin_sb[:, bsl])
        nc.vector.tensor_add(out_t[:, bsl], m1[:], t2s[:])

    def qk_chunk(which, pair, blk):
        qk_phase2(which, pair, blk, qk_phase1(which, pair, blk))

    def v_chunk(jc):
        """V for all 8 heads at j-chunk jc, bias added, ones column."""
        vp = qkps.tile([128, DG], F32, tag="qk", name="vp")
        for kc in range(KC):
            nc.tensor.matmul(
                vp[:], xt_sb[:, kc, jc * 128:(jc + 1) * 128], wv_sb[:, kc, :],
                start=(kc == 0), stop=False)
        nc.tensor.matmul(vp[:], ones128, bv_sb[:], start=False, stop=True)
        vv = vaug[jc][:].rearrange("p (h c) -> p h c", h=HPC)
        nc.vector.tensor_copy(
            vv[:, :, HD:HD + 1],
            ones_sb[:, 0:HPC].rearrange("p (h one) -> p h one", one=1))
        nc.vector.tensor_copy(
            vv[:, :, 0:HD], vp[:].rearrange("p (h c) -> p h c", h=HPC))

    def unit(pair, ib, sched):
        """Attention for head pair `pair`, queries [ib*512, (ib+1)*512).

        One call per j-chunk: two row-tiled scores matmuls (heads A/B) into
        a 2-bank PSUM tile (pool bufs=2 double-buffers the ring), one
        ACTIVATE exp [128, 1024], then the lag-1 ctx matmuls for j-1.
        `sched` maps call index -> list of filler thunks (QKV production).
        """
        isl = slice(ib * IB, (ib + 1) * IB)
        ctx_ps = [cps.tile([HD + 1, IB], F32, tag="ctx", name=f"ctx{h}")
                  for h in range(2)]
        pend = []

        def emit_ctx():
            p, j = pend.pop(0)
            for h in range(2):
                hh = 2 * pair + h
                nc.tensor.matmul(
                    ctx_ps[h][:],
                    vaug[j][:, hh * (HD + 1):(hh + 1) * (HD + 1)],
                    p[:, h * IB:(h + 1) * IB],
                    start=(j == 0), stop=(j == NJ - 1))

        for j in range(NJ):
            sc = scps.tile([128, 2 * IB], F32, tag="sc", name="sc")
            for h in range(2):
                hp = slice(h * 64, (h + 1) * 64)
                nc.tensor.matmul(
                    sc[:, h * IB:(h + 1) * IB],
                    kt[pair][hp, j * 128:(j + 1) * 128], qt[pair][hp, isl],
                    start=True, stop=True)
            p = ppool.tile([128, 2 * IB], BF16, tag="p", name="p")
            nc.scalar.activation(p[:], sc[:], EXP, scale=1.0 / 65536.0)
            pend.append((p, j))
            for th in sched.get(j, ()):
                th()
            # batch ctx two j-chunks at a time: fewer PE tile-mode switches
            if j >= 3 and j % 2 == 1:
                emit_ctx()
                emit_ctx()
        for th in sched.get(NJ, ()):
            th()
        emit_ctx()
        emit_ctx()
        assert not pend
        for h in range(2):
            ca = capool.tile([HD + 1, IB], F32, tag="ca")
            nc.vector.tensor_copy(ca[:], ctx_ps[h][:])
            # custom DVE ops don't handle a shifted partition base: copy the
            # den row down to partition 0 before the reciprocal
            dn = rpool.tile([1, IB], F32, tag="dn")
            nc.vector.tensor_copy(dn[:], ca[HD:HD + 1, :])
            rec = rpool.tile([1, IB], F32, tag="rec")
            nc.vector.reciprocal_approx_fast(rec[:], dn[:])
            rbc = bpool.tile([HD, IB], F32, tag="rbc")
            nc.gpsimd.partition_broadcast(rbc[:], rec[:], channels=HD)
            ctxn = npool.tile([HD, IB], F32, tag="ctxn")
            nc.vector.tensor_mul(ctxn[:], ca[0:HD, :], rbc[:])
            hh = 2 * pair + h
            nc.sync.dma_start(o_d[hh * HD:(hh + 1) * HD, isl], ctxn[:])

    # minimal prefix: just enough for unit (pair0, ib0) to start streaming;
    # everything else is produced inside unit call slots
    v_chunk(0)
    v_chunk(1)
    qk_chunk("k", 0, 0)
    qk_chunk("q", 0, 0)

    def TH(*a):
        return lambda: qk_chunk(*a)

    def TV(jc):
        return lambda: v_chunk(jc)

    for pair in range(NPAIR):
        for ib in range(NIB):
            sched = {}
            if pair == 0 and ib == 0:
                sched = {0: [TV(2), TV(3)], 1: [TH("k", 0, 1), TV(4)],
                         2: [TV(5)], 3: [TV(6), TH("k", 0, 2)],
                         4: [TV(7)], 5: [TV(8)], 6: [TV(9)],
                         7: [TH("k", 0, 3), TV(10)], 8: [TV(11)],
                         9: [TV(12)], 10: [TV(13)], 11: [TV(14)],
                         12: [TV(15)], 13: [TH("q", 0, 1)]}
            elif ib == 0:
                sched = {1: [TH("k", pair, 1)], 3: [TH("k", pair, 2)],
                         7: [TH("k", pair, 3)], 10: [TH("q", pair, 1)]}
            elif ib == 1:
                sched = {5: [TH("q", pair, 2)]}
            elif ib == 2:
                sched = {5: [TH("q", pair, 3)]}
            elif ib == 3 and pair + 1 < NPAIR:
                sched = {4: [TH("k", pair + 1, 0)],
                         8: [TH("q", pair + 1, 0)]}
            unit(pair, ib, sched)

def _build():
    if "nc" in _CACHE:
        return _CACHE["nc"]
    nc = bacc.Bacc("TRN2", target_bir_lowering=False, debug=False,
                   num_devices=NCORES)
    names_shapes = [
        ("xt", [H, S], BF16), ("xt8", [H, S], F8),
        ("wq8", [H, DG], F8), ("wk8", [H, DG], F8),
        ("wv", [H, DG], BF16),
        ("bq", [128, NPAIR], F32), ("bk", [128, NPAIR], F32),
        ("bv", [1, DG], BF16),
        ("cos", [128, S], BF16), ("sin", [128, S], F32),
        ("ones", [128, IB], BF16), ("rsw", [128, 128], BF16),
    ]
    ins = [nc.dram_tensor(n, s, dt, kind="ExternalInput").ap()
           for n, s, dt in names_shapes]
    o_d = nc.dram_tensor("o", [DG, S], F32, kind="ExternalOutput").ap()
    from contextlib import ExitStack
    with tile.TileContext(nc) as tc:
        with ExitStack() as ctx:
            _emit(nc, tc, ctx, ins, o_d)
    nc.compile()
    _CACHE["nc"] = nc
    return nc


def _rotary_tables():
    inv_freq = (1.0 / (10000.0 ** (np.arange(0, HD, 2, dtype=np.float32)
                                   / np.float32(HD)))).astype(np.float32)
    t = np.arange(S, dtype=np.float32)
    freqs = np.outer(t, inv_freq).astype(np.float32)       # [S, 32]
    emb = np.concatenate([freqs, freqs], axis=-1)          # [S, 64]
    cos_t = np.cos(emb).T.astype(np.float32)               # [64, S]
    sin_t = np.sin(emb).T.astype(np.float32)               # unsigned
    cos2 = np.ascontiguousarray(np.concatenate([cos_t, cos_t], axis=0))
    sin2 = np.ascontiguousarray(np.concatenate([sin_t, sin_t], axis=0))
    # signed rotate-half permutation: t2_pre[d] = sign(d) * q[swap(d)],
    # sign = -1 on first half of each 64-block
    rsw = np.zeros((128, 128), dtype=np.float32)
    for d in range(128):
        blk, dd = d // 64, d % 64
        src = blk * 64 + (dd + 32) % 64
        rsw[src, d] = -1.0 if dd < 32 else 1.0
    return cos2, sin2, rsw


def _in_maps(hidden_states, Wq, bq, Wk, bk, Wv, bv):
    cos2, sin2, rsw = _rotary_tables()
    xts = [np.ascontiguousarray(hidden_states[b].T).astype(NPBF16)
           for b in range(B)]
    xt8s = [np.ascontiguousarray(hidden_states[b].T).astype(NPF8)
            for b in range(B)]
    w_slices = {}
    for g in range(2):
        dsl = slice(g * DG, (g + 1) * DG)
        w_slices[g] = dict(
            wq8=np.ascontiguousarray(Wq[:, dsl] * 32.0).astype(NPF8),
            wk8=np.ascontiguousarray(Wk[:, dsl] * 32.0).astype(NPF8),
            wv=np.ascontiguousarray(Wv[:, dsl]).astype(NPBF16),
            bq=np.ascontiguousarray(bq[dsl].reshape(NPAIR, 128).T) * 32.0,
            bk=np.ascontiguousarray(bk[dsl].reshape(NPAIR, 128).T) * 32.0,
            bv=np.ascontiguousarray(bv[dsl].reshape(1, DG)).astype(NPBF16),
        )
    onesm = np.ones((128, IB), dtype=NPBF16)
    maps = []
    for c in range(NCORES):
        b, g = c // 2, c % 2
        m = {"xt": xts[b], "xt8": xt8s[b], "cos": cos2.astype(NPBF16),
             "sin": sin2, "ones": onesm, "rsw": rsw.astype(NPBF16)}
        m.update(w_slices[g])
        maps.append(m)
    return maps


def run(inputs, **kw):
    inputs = {k: np.asarray(v, dtype=np.float32) for k, v in inputs.items()}
    nc = _build()
    maps = _in_maps(**inputs)
    try:
        res = bass_utils.run_bass_kernel_spmd(
            nc, maps, core_ids=list(range(NCORES)), **kw)
    except Exception:
        # transient device errors (e.g. NRT_EXEC_UNIT_UNRECOVERABLE) clear on
        # retry
        res = bass_utils.run_bass_kernel_spmd(
            nc, maps, core_ids=list(range(NCORES)), **kw)
    out = np.empty((B, S, H), dtype=np.float32)
    for c in range(NCORES):
        b, g = c // 2, c % 2
        out[b, :, g * DG:(g + 1) * DG] = res.results[c]["o"].T
    return out, res


def kernel(**inputs):
    out, _ = run(inputs)
    return out
